# revision 36
# baseline (speedup 1.0000x reference)
"""Trainium2 Bass kernel for nn_HardQuadTripletSOSRLoss.

Sharding: 8 cores = 2 batches x 4 HW-shards (4096 grid cells each).

Device work per core (b, s) — the dominant retrieval stage (>97% of FLOPs):
  - PE: bf16 matmuls, dsim scores kp1_desc[b] @ desc2f[b, shard]^T as
    4 row-tiles x 2 chunks of 2048 cols (4 x 512-col PSUM banks each),
    with warm-up matmuls to ramp the PE p-state while DMAs land.
  - ACT: drains each PSUM chunk to SBUF as fp16 (the conversion enables
    the DVE's packed 2-elem/cycle mode downstream).
  - DVE: two packed pairwise-max halvings (2048->1024->512) + max8 ->
    top-8-of-quads candidate values per chunk.
Host: bilinear descriptor sampling, grid geometry, masks, the small
O(N^2 C) k_sim/w_sim SOS stage, candidate merge with an exactness
certificate + per-row repair, final loss.
"""

import numpy as np
import ml_dtypes

import concourse.bass as bass
import concourse.mybir as mybir
import concourse.tile as tile
from concourse import bacc
from concourse.bass_utils import run_bass_kernel_spmd

# ---- problem constants (hardcoded per contract) ----
B, N, C, H, W = 2, 512, 128, 128, 128
HW = H * W
GS = 8
NUM_NEG = 16
SOS_NEG = 8
MARGIN = 1.0
NSHARD = 4
SHW = HW // NSHARD          # 4096 cells per shard
RT = N // 128               # 4 row tiles
CHUNK = 2048
NCH = SHW // CHUNK          # 2 chunks per row-tile

F32 = mybir.dt.float32
F16 = mybir.dt.float16
BF16 = mybir.dt.bfloat16
BF = ml_dtypes.bfloat16

_NC_CACHE = {}
LAST_RESULTS = None  # BassKernelResults of most recent device run (for test.py)


def _build_nc():
    nc = bacc.Bacc("TRN2", target_bir_lowering=False, debug=False, num_devices=8)

    lhsT = nc.dram_tensor("lhsT", [C, N], BF16, kind="ExternalInput")
    rhs = nc.dram_tensor("rhs", [C, SHW], BF16, kind="ExternalInput")
    cand = nc.dram_tensor("cand", [128, RT * NCH * 8], F16, kind="ExternalOutput")

    with tile.TileContext(nc) as tc:
        with (
            tc.tile_pool(name="const", bufs=1) as cpool,
            tc.tile_pool(name="cv", bufs=3) as cvpool,
            tc.tile_pool(name="h1", bufs=3) as h1pool,
            tc.tile_pool(name="h2", bufs=3) as h2pool,
            tc.tile_pool(name="psum", bufs=2, space="PSUM") as pspool,
        ):
            warm = cpool.tile([128, 512], BF16, tag="warm")
            nc.vector.memset(warm[:], 0)
            lhsT_sb = cpool.tile([C, N], BF16, tag="lhsT")
            rhs_sb = cpool.tile([C, SHW], BF16, tag="rhs")
            # wave 1: lhsT + row-tile chunk 0's cells, enough parallel streams
            # to saturate the HBM share; wave 2 (chunk 1's cells) is held back
            # by a gate copy whose strided read spans wave-1 columns (RAW:
            # wait for them) and wave-2 dst columns (WAR: those DMAs wait), so
            # the first chunk gets the full bandwidth and the ACT spine starts
            # ~3us earlier.
            nc.sync.dma_start(lhsT_sb[:], lhsT[:, :])
            nc.sync.dma_start(rhs_sb[:, 0:512], rhs[:, 0:512])
            nc.sync.dma_start(rhs_sb[:, 512:1024], rhs[:, 512:1024])
            nc.scalar.dma_start(rhs_sb[:, 1024:1536], rhs[:, 1024:1536])
            nc.scalar.dma_start(rhs_sb[:, 1536:2048], rhs[:, 1536:2048])
            gate = cpool.tile([128, 8], BF16, tag="gate")
            nc.gpsimd.tensor_copy(gate[:], rhs_sb[:, 511:4096:512])
            for lo, hi in ((2048, 2560), (2560, 3072), (3072, 3584), (3584, 4096)):
                nc.gpsimd.dma_start(rhs_sb[:, lo:hi], rhs[:, lo:hi])

            cnd = cpool.tile([128, RT * NCH * 8], F16, tag="cnd")
            MX = mybir.AluOpType.max

            # PE p-state warm-up on zeros while the real inputs stream in
            Gw = pspool.tile([128, 2048], F32, tag="ps")
            for _ in range(5):
                nc.tensor.matmul(
                    Gw[:, 0:512], warm[:, 0:128], warm[:], start=True, stop=True
                )

            for i in range(RT * NCH):
                rt, ch = divmod(i, NCH)
                G = pspool.tile([128, 2048], F32, tag="ps")
                for q in range(4):
                    nc.tensor.matmul(
                        G[:, q * 512 : (q + 1) * 512],
                        lhsT_sb[:, rt * 128 : (rt + 1) * 128],
                        rhs_sb[:, ch * 2048 + q * 512 : ch * 2048 + (q + 1) * 512],
                        start=True,
                        stop=True,
                    )
                cv = cvpool.tile([128, 2048], F16, tag="cv")
                nc.scalar.copy(cv[:], G[:])
                h1 = h1pool.tile([128, 1024], F16, tag="h1")
                nc.vector.tensor_tensor(h1[:], cv[:, 0:1024], cv[:, 1024:2048], MX)
                h2 = h2pool.tile([128, 512], F16, tag="h2")
                nc.vector.tensor_tensor(h2[:], h1[:, 0:512], h1[:, 512:1024], MX)
                nc.vector.max(cnd[:, i * 8 : (i + 1) * 8], h2[:])

            nc.sync.dma_start(cand[:, :], cnd[:])

    nc.compile()
    return nc


def _get_nc():
    if "nc" not in _NC_CACHE:
        _NC_CACHE["nc"] = _build_nc()
    return _NC_CACHE["nc"]


# ---------------- host-side helpers (all float32, mirror reference) ----------


def _sample_descriptors(desc2, kp):
    """Bilinear sample of desc2 (B,C,H,W) at image-space (y,x) kp, L2-normed."""
    b, c, h, w = desc2.shape
    f = np.float32
    y = np.clip(kp[..., 0] / f(GS) - f(0.5), f(0.0), f(h - 1.0)).astype(f)
    x = np.clip(kp[..., 1] / f(GS) - f(0.5), f(0.0), f(w - 1.0)).astype(f)
    y0 = np.clip(np.floor(y), 0, h - 2).astype(np.int64)
    x0 = np.clip(np.floor(x), 0, w - 2).astype(np.int64)
    wy = (y - y0.astype(f))[..., None]
    wx = (x - x0.astype(f))[..., None]
    dmap = desc2.transpose(0, 2, 3, 1).reshape(b, h * w, c)

    def g(yi, xi):
        idx = yi * w + xi
        return np.take_along_axis(dmap, idx[..., None], axis=1)

    v = (
        g(y0, x0) * (1 - wy) * (1 - wx)
        + g(y0, x0 + 1) * (1 - wy) * wx
        + g(y0 + 1, x0) * wy * (1 - wx)
        + g(y0 + 1, x0 + 1) * wy * wx
    )
    n = np.sqrt(np.sum(v * v, axis=-1, keepdims=True)).astype(f)
    return (v / (n + f(1e-8))).astype(f)


def _nearest4(pts):
    """Flat ids (..., 4) of the 4 nearest grid-cell centers, matching the
    reference's top_k over all HW cells (ties -> lower flat id)."""
    f = np.float32
    y = pts[..., 0]
    x = pts[..., 1]
    cy = np.clip(np.floor(y / f(GS)).astype(np.int64), 0, H - 1)
    cx = np.clip(np.floor(x / f(GS)).astype(np.int64), 0, W - 1)
    by = np.clip(cy - 2, 0, H - 5)
    bx = np.clip(cx - 2, 0, W - 5)
    offs = np.arange(5, dtype=np.int64)
    iy = by[..., None] + offs          # (..., 5)
    ix = bx[..., None] + offs
    cyc = (f(GS) * iy + f(GS / 2.0)).astype(f)
    cxc = (f(GS) * ix + f(GS / 2.0)).astype(f)
    dy = y[..., None] - cyc
    dx = x[..., None] - cxc
    d2 = (dy * dy)[..., :, None] + (dx * dx)[..., None, :]   # (..., 5, 5)
    ids = iy[..., :, None] * W + ix[..., None, :]
    d2 = d2.reshape(d2.shape[:-2] + (25,))
    ids = ids.reshape(ids.shape[:-2] + (25,))
    # candidates are flat-id ascending, so a stable sort on d2 reproduces
    # top_k's lower-index tie-break
    order = np.argsort(d2, axis=-1, kind="stable")[..., :4]
    return np.take_along_axis(ids, order, axis=-1)


def _warp(p, Hm):
    f = np.float32
    xy = p[..., ::-1]
    ph = np.concatenate([xy, np.ones_like(xy[..., :1])], axis=-1)
    wp = np.einsum("bij,bmj->bmi", Hm, ph).astype(f)
    wp = wp[..., :2] / (wp[..., 2:3] + f(1e-8))
    return wp[..., ::-1].astype(f)


def _centers(ids):
    f = np.float32
    yy = (ids // W).astype(f) * f(GS) + f(GS / 2.0)
    xx = (ids % W).astype(f) * f(GS) + f(GS / 2.0)
    return np.stack([yy, xx], axis=-1)


def kernel(kp1, w_kp1, kp1_desc, desc2, homo12):
    global LAST_RESULTS
    import os

    f = np.float32
    kp1 = np.asarray(kp1, f)
    w_kp1 = np.asarray(w_kp1, f)
    kp1_desc = np.asarray(kp1_desc, f)
    desc2 = np.asarray(desc2, f)
    homo12 = np.asarray(homo12, f)

    # ---------------- host geometry / small tensors ----------------
    w_kp1_desc = _sample_descriptors(desc2, w_kp1)                  # (B,N,C)
    pos = f(2.0) - f(2.0) * np.einsum("bnc,bnc->bn", kp1_desc, w_kp1_desc)

    cell4 = _nearest4(kp1)                                          # (B,N,4)
    kp1_cells = _centers(cell4.reshape(B, 4 * N))                   # (B,4N,2)
    warped = _warp(kp1_cells, homo12)                               # (B,4N,2)
    wcc = _nearest4(warped)                                         # (B,4N,4)
    ids16 = wcc.reshape(B, N, 16)                                   # neigh cells
    cell4_w = _nearest4(w_kp1)                                      # (B,N,4)

    # kp1_mask[n,n'] = #coinciding cells between cell4[n] and cell4[n']
    eqk = cell4[:, :, :, None, None] == cell4[:, None, None, :, :]
    kp1_mask = eqk.sum(axis=(2, 4)).astype(f)                       # (B,N,N)
    # w_kp1_mask[n,n'] = #coincidences between ids16[n] and cell4_w[n']
    eqw = ids16[:, :, :, None, None] == cell4_w[:, None, None, :, :]
    w_kp1_mask = eqw.sum(axis=(2, 4)).astype(f)                     # (B,N,N)

    # ---------------- device run (dsim retrieval) ----------------
    nc = _get_nc()
    in_maps = []
    desc2_flat = np.ascontiguousarray(desc2.reshape(B, C, HW))
    for b in range(B):
        lhsT_b = np.ascontiguousarray(kp1_desc[b].T.astype(BF))
        for s in range(NSHARD):
            in_maps.append(
                {
                    "lhsT": lhsT_b,
                    "rhs": np.ascontiguousarray(
                        desc2_flat[b][:, s * SHW : (s + 1) * SHW].astype(BF)
                    ),
                }
            )
    want_trace = bool(int(os.environ.get("KT_TRACE", "0")))
    try:
        res = run_bass_kernel_spmd(
            nc, in_maps, core_ids=list(range(8)), trace=want_trace
        )
    except ModuleNotFoundError:
        res = run_bass_kernel_spmd(nc, in_maps, core_ids=list(range(8)), trace=False)
    LAST_RESULTS = res
    results = res.results

    # cand_all[b, n, chunk(8 per batch-row), 8]; chunk s*2+ch covers shard-s
    # cells [ch*2048, (ch+1)*2048)
    NCHB = NSHARD * NCH                                             # 8
    cand_all = np.empty((B, N, NCHB, 8), f)
    for ci, (b, s) in enumerate((b, s) for b in range(B) for s in range(NSHARD)):
        cm = np.asarray(results[ci]["cand"], np.float32)            # (128, 64)
        for rt in range(RT):
            for ch in range(NCH):
                i = rt * NCH + ch
                cand_all[b, rt * 128 : (rt + 1) * 128, s * NCH + ch, :] = cm[
                    :, i * 8 : (i + 1) * 8
                ]

    # ---------------- fos: merge per-chunk candidates ----------------
    flat = cand_all.reshape(B, N, NCHB * 8)
    chunk_min = cand_all[..., 7]                                    # (B,N,8)
    srt = np.sort(flat, axis=-1)[..., ::-1]                         # desc
    thr32 = srt[..., 31]
    CERT_EPS = f(2e-3)
    bad = (chunk_min >= thr32[..., None] - CERT_EPS).any(axis=-1)

    # host raw scores of masked cells (for value-matched patching)
    hwdesc = desc2_flat.transpose(0, 2, 1)                          # (B,HW,C)
    gath = np.take_along_axis(
        hwdesc, ids16.reshape(B, N * 16)[:, :, None], axis=1
    ).reshape(B, N, 16, C)
    vm16 = np.einsum("bnc,bnjc->bnj", kp1_desc, gath).astype(f)     # (B,N,16)

    TOL = 2e-3
    PATCH_W = 48
    neg_scores = np.empty((B, N, NUM_NEG), f)
    repair = []
    for b in range(B):
        for n in range(N):
            if bad[b, n]:
                repair.append((b, n))
                continue
            cv = srt[b, n, :PATCH_W].copy()
            uq, inv, cnts = np.unique(
                ids16[b, n], return_index=True, return_counts=True
            )
            vms = vm16[b, n][inv]
            lo = cv[-1] - TOL
            ok = True
            for v, cnt in zip(vms, cnts):
                if v < lo:
                    continue
                j = np.argmin(np.abs(cv - v))
                if abs(cv[j] - v) > TOL:
                    ok = False
                    break
                cv[j] -= f(2.5) * cnt
            if not ok:
                repair.append((b, n))
                continue
            merged = np.sort(np.concatenate([cv, srt[b, n, PATCH_W:]]))[::-1]
            neg_scores[b, n] = merged[:NUM_NEG]

    if repair:
        for b, n in repair:
            row = hwdesc[b] @ kp1_desc[b, n]                        # (HW,)
            np.subtract.at(row, ids16[b, n], f(2.5))
            neg_scores[b, n] = np.sort(row)[::-1][:NUM_NEG]

    neg = f(2.0) - f(2.0) * neg_scores                              # ascending dsim
    fos = np.mean(
        np.maximum(pos[..., None] - neg + f(MARGIN), f(0.0)) ** 2
    ).astype(f)

    # ---------------- sos (exact, host: O(N^2 C) ~ 3% of total FLOPs) ----
    def top8_ids(desc, mask):
        out = np.empty((B, N, SOS_NEG), np.int64)
        for b in range(B):
            sim = f(2.0) - f(2.0) * (desc[b] @ desc[b].T) + f(5.0) * mask[b]
            out[b] = np.argsort(sim, axis=-1, kind="stable")[:, :SOS_NEG]
        return out

    k_idsF = top8_ids(kp1_desc, kp1_mask)
    w_idsF = top8_ids(w_kp1_desc, w_kp1_mask)

    kd = np.take_along_axis(
        kp1_desc, k_idsF.reshape(B, N * 8)[:, :, None], axis=1
    ).reshape(B, N, 8, C)
    wd = np.take_along_axis(
        w_kp1_desc, w_idsF.reshape(B, N * 8)[:, :, None], axis=1
    ).reshape(B, N, 8, C)
    a = f(2.0) - f(2.0) * np.einsum("bnc,bnkc->bnk", kp1_desc, kd)
    bb = f(2.0) - f(2.0) * np.einsum("bnc,bnkc->bnk", w_kp1_desc, wd)
    sv = (a - bb).astype(f)
    sos = np.mean(np.sqrt(np.sum(sv * sv, axis=-1))).astype(f)

    return np.asarray(fos + sos, dtype=np.float32)


# revision 37
# speedup vs baseline: 1.0309x; 1.0309x over previous
"""Trainium2 Bass kernel for nn_HardQuadTripletSOSRLoss.

Sharding: 8 cores = 2 batches x 4 HW-shards (4096 grid cells each).

Device work per core (b, s) — the dominant retrieval stage (>97% of FLOPs):
  - PE: bf16 matmuls, dsim scores kp1_desc[b] @ desc2f[b, shard]^T as
    4 row-tiles x 2 chunks of 2048 cols (4 x 512-col PSUM banks each),
    with warm-up matmuls to ramp the PE p-state while DMAs land.
  - ACT: drains each PSUM chunk to SBUF as fp16 (the conversion enables
    the DVE's packed 2-elem/cycle mode downstream).
  - DVE: two packed pairwise-max halvings (2048->1024->512) + max8 ->
    top-8-of-quads candidate values per chunk.
Host: bilinear descriptor sampling, grid geometry, masks, the small
O(N^2 C) k_sim/w_sim SOS stage, candidate merge with an exactness
certificate + per-row repair, final loss.
"""

import numpy as np
import ml_dtypes

import concourse.bass as bass
import concourse.mybir as mybir
import concourse.tile as tile
from concourse import bacc
from concourse.bass_utils import run_bass_kernel_spmd

# ---- problem constants (hardcoded per contract) ----
B, N, C, H, W = 2, 512, 128, 128, 128
HW = H * W
GS = 8
NUM_NEG = 16
SOS_NEG = 8
MARGIN = 1.0
NSHARD = 4
SHW = HW // NSHARD          # 4096 cells per shard
RT = N // 128               # 4 row tiles
CHUNK = 2048
NCH = SHW // CHUNK          # 2 chunks per row-tile

F32 = mybir.dt.float32
F16 = mybir.dt.float16
BF16 = mybir.dt.bfloat16
BF = ml_dtypes.bfloat16

_NC_CACHE = {}
LAST_RESULTS = None  # BassKernelResults of most recent device run (for test.py)


def _build_nc():
    nc = bacc.Bacc("TRN2", target_bir_lowering=False, debug=False, num_devices=8)

    lhsT = nc.dram_tensor("lhsT", [C, N], BF16, kind="ExternalInput")
    rhs = nc.dram_tensor("rhs", [C, SHW], BF16, kind="ExternalInput")
    cand = nc.dram_tensor("cand", [128, RT * NCH * 8], F16, kind="ExternalOutput")

    with tile.TileContext(nc) as tc:
        with (
            tc.tile_pool(name="const", bufs=1) as cpool,
            tc.tile_pool(name="cv", bufs=3) as cvpool,
            tc.tile_pool(name="h1", bufs=3) as h1pool,
            tc.tile_pool(name="h2", bufs=3) as h2pool,
            tc.tile_pool(name="psum", bufs=2, space="PSUM") as pspool,
        ):
            warm = cpool.tile([128, 512], BF16, tag="warm")
            nc.vector.memset(warm[:], 0)
            lhsT_sb = cpool.tile([C, N], BF16, tag="lhsT")
            rhs_sb = cpool.tile([C, SHW], BF16, tag="rhs")
            # spread streams over the idle DGE queues; chunk0's four 512-col
            # streams match its four matmuls so they start as data arrives
            nc.sync.dma_start(lhsT_sb[:], lhsT[:, :])
            nc.sync.dma_start(rhs_sb[:, 0:512], rhs[:, 0:512])
            nc.sync.dma_start(rhs_sb[:, 512:1024], rhs[:, 512:1024])
            nc.scalar.dma_start(rhs_sb[:, 1024:1536], rhs[:, 1024:1536])
            nc.scalar.dma_start(rhs_sb[:, 1536:2048], rhs[:, 1536:2048])
            for lo, hi in ((2048, 2560), (2560, 3072), (3072, 3584), (3584, 4096)):
                nc.gpsimd.dma_start(rhs_sb[:, lo:hi], rhs[:, lo:hi])

            cnd = cpool.tile([128, RT * NCH * 8], F16, tag="cnd")
            MX = mybir.AluOpType.max

            # PE p-state warm-up on zeros while the real inputs stream in
            Gw = pspool.tile([128, 2048], F32, tag="ps")
            for _ in range(5):
                nc.tensor.matmul(
                    Gw[:, 0:512], warm[:, 0:128], warm[:], start=True, stop=True
                )

            for i in range(RT * NCH):
                rt, ch = divmod(i, NCH)
                G = pspool.tile([128, 2048], F32, tag="ps")
                for q in range(4):
                    nc.tensor.matmul(
                        G[:, q * 512 : (q + 1) * 512],
                        lhsT_sb[:, rt * 128 : (rt + 1) * 128],
                        rhs_sb[:, ch * 2048 + q * 512 : ch * 2048 + (q + 1) * 512],
                        start=True,
                        stop=True,
                    )
                cv = cvpool.tile([128, 2048], F16, tag="cv")
                nc.scalar.copy(cv[:], G[:])
                h1 = h1pool.tile([128, 1024], F16, tag="h1")
                nc.vector.tensor_tensor(h1[:], cv[:, 0:1024], cv[:, 1024:2048], MX)
                h2 = h2pool.tile([128, 512], F16, tag="h2")
                nc.vector.tensor_tensor(h2[:], h1[:, 0:512], h1[:, 512:1024], MX)
                nc.vector.max(cnd[:, i * 8 : (i + 1) * 8], h2[:])

            nc.sync.dma_start(cand[:, :], cnd[:])

    nc.compile()
    return nc


def _get_nc():
    if "nc" not in _NC_CACHE:
        _NC_CACHE["nc"] = _build_nc()
    return _NC_CACHE["nc"]


# ---------------- host-side helpers (all float32, mirror reference) ----------


def _sample_descriptors(desc2, kp):
    """Bilinear sample of desc2 (B,C,H,W) at image-space (y,x) kp, L2-normed."""
    b, c, h, w = desc2.shape
    f = np.float32
    y = np.clip(kp[..., 0] / f(GS) - f(0.5), f(0.0), f(h - 1.0)).astype(f)
    x = np.clip(kp[..., 1] / f(GS) - f(0.5), f(0.0), f(w - 1.0)).astype(f)
    y0 = np.clip(np.floor(y), 0, h - 2).astype(np.int64)
    x0 = np.clip(np.floor(x), 0, w - 2).astype(np.int64)
    wy = (y - y0.astype(f))[..., None]
    wx = (x - x0.astype(f))[..., None]
    dmap = desc2.transpose(0, 2, 3, 1).reshape(b, h * w, c)

    def g(yi, xi):
        idx = yi * w + xi
        return np.take_along_axis(dmap, idx[..., None], axis=1)

    v = (
        g(y0, x0) * (1 - wy) * (1 - wx)
        + g(y0, x0 + 1) * (1 - wy) * wx
        + g(y0 + 1, x0) * wy * (1 - wx)
        + g(y0 + 1, x0 + 1) * wy * wx
    )
    n = np.sqrt(np.sum(v * v, axis=-1, keepdims=True)).astype(f)
    return (v / (n + f(1e-8))).astype(f)


def _nearest4(pts):
    """Flat ids (..., 4) of the 4 nearest grid-cell centers, matching the
    reference's top_k over all HW cells (ties -> lower flat id)."""
    f = np.float32
    y = pts[..., 0]
    x = pts[..., 1]
    cy = np.clip(np.floor(y / f(GS)).astype(np.int64), 0, H - 1)
    cx = np.clip(np.floor(x / f(GS)).astype(np.int64), 0, W - 1)
    by = np.clip(cy - 2, 0, H - 5)
    bx = np.clip(cx - 2, 0, W - 5)
    offs = np.arange(5, dtype=np.int64)
    iy = by[..., None] + offs          # (..., 5)
    ix = bx[..., None] + offs
    cyc = (f(GS) * iy + f(GS / 2.0)).astype(f)
    cxc = (f(GS) * ix + f(GS / 2.0)).astype(f)
    dy = y[..., None] - cyc
    dx = x[..., None] - cxc
    d2 = (dy * dy)[..., :, None] + (dx * dx)[..., None, :]   # (..., 5, 5)
    ids = iy[..., :, None] * W + ix[..., None, :]
    d2 = d2.reshape(d2.shape[:-2] + (25,))
    ids = ids.reshape(ids.shape[:-2] + (25,))
    # candidates are flat-id ascending, so a stable sort on d2 reproduces
    # top_k's lower-index tie-break
    order = np.argsort(d2, axis=-1, kind="stable")[..., :4]
    return np.take_along_axis(ids, order, axis=-1)


def _warp(p, Hm):
    f = np.float32
    xy = p[..., ::-1]
    ph = np.concatenate([xy, np.ones_like(xy[..., :1])], axis=-1)
    wp = np.einsum("bij,bmj->bmi", Hm, ph).astype(f)
    wp = wp[..., :2] / (wp[..., 2:3] + f(1e-8))
    return wp[..., ::-1].astype(f)


def _centers(ids):
    f = np.float32
    yy = (ids // W).astype(f) * f(GS) + f(GS / 2.0)
    xx = (ids % W).astype(f) * f(GS) + f(GS / 2.0)
    return np.stack([yy, xx], axis=-1)


def kernel(kp1, w_kp1, kp1_desc, desc2, homo12):
    global LAST_RESULTS
    import os

    f = np.float32
    kp1 = np.asarray(kp1, f)
    w_kp1 = np.asarray(w_kp1, f)
    kp1_desc = np.asarray(kp1_desc, f)
    desc2 = np.asarray(desc2, f)
    homo12 = np.asarray(homo12, f)

    # ---------------- host geometry / small tensors ----------------
    w_kp1_desc = _sample_descriptors(desc2, w_kp1)                  # (B,N,C)
    pos = f(2.0) - f(2.0) * np.einsum("bnc,bnc->bn", kp1_desc, w_kp1_desc)

    cell4 = _nearest4(kp1)                                          # (B,N,4)
    kp1_cells = _centers(cell4.reshape(B, 4 * N))                   # (B,4N,2)
    warped = _warp(kp1_cells, homo12)                               # (B,4N,2)
    wcc = _nearest4(warped)                                         # (B,4N,4)
    ids16 = wcc.reshape(B, N, 16)                                   # neigh cells
    cell4_w = _nearest4(w_kp1)                                      # (B,N,4)

    # kp1_mask[n,n'] = #coinciding cells between cell4[n] and cell4[n']
    eqk = cell4[:, :, :, None, None] == cell4[:, None, None, :, :]
    kp1_mask = eqk.sum(axis=(2, 4)).astype(f)                       # (B,N,N)
    # w_kp1_mask[n,n'] = #coincidences between ids16[n] and cell4_w[n']
    eqw = ids16[:, :, :, None, None] == cell4_w[:, None, None, :, :]
    w_kp1_mask = eqw.sum(axis=(2, 4)).astype(f)                     # (B,N,N)

    # ---------------- device run (dsim retrieval) ----------------
    nc = _get_nc()
    in_maps = []
    desc2_flat = np.ascontiguousarray(desc2.reshape(B, C, HW))
    for b in range(B):
        lhsT_b = np.ascontiguousarray(kp1_desc[b].T.astype(BF))
        for s in range(NSHARD):
            in_maps.append(
                {
                    "lhsT": lhsT_b,
                    "rhs": np.ascontiguousarray(
                        desc2_flat[b][:, s * SHW : (s + 1) * SHW].astype(BF)
                    ),
                }
            )
    want_trace = bool(int(os.environ.get("KT_TRACE", "0")))
    try:
        res = run_bass_kernel_spmd(
            nc, in_maps, core_ids=list(range(8)), trace=want_trace
        )
    except ModuleNotFoundError:
        res = run_bass_kernel_spmd(nc, in_maps, core_ids=list(range(8)), trace=False)
    LAST_RESULTS = res
    results = res.results

    # cand_all[b, n, chunk(8 per batch-row), 8]; chunk s*2+ch covers shard-s
    # cells [ch*2048, (ch+1)*2048)
    NCHB = NSHARD * NCH                                             # 8
    cand_all = np.empty((B, N, NCHB, 8), f)
    for ci, (b, s) in enumerate((b, s) for b in range(B) for s in range(NSHARD)):
        cm = np.asarray(results[ci]["cand"], np.float32)            # (128, 64)
        for rt in range(RT):
            for ch in range(NCH):
                i = rt * NCH + ch
                cand_all[b, rt * 128 : (rt + 1) * 128, s * NCH + ch, :] = cm[
                    :, i * 8 : (i + 1) * 8
                ]

    # ---------------- fos: merge per-chunk candidates ----------------
    flat = cand_all.reshape(B, N, NCHB * 8)
    chunk_min = cand_all[..., 7]                                    # (B,N,8)
    srt = np.sort(flat, axis=-1)[..., ::-1]                         # desc
    thr32 = srt[..., 31]
    CERT_EPS = f(2e-3)
    bad = (chunk_min >= thr32[..., None] - CERT_EPS).any(axis=-1)

    # host raw scores of masked cells (for value-matched patching)
    hwdesc = desc2_flat.transpose(0, 2, 1)                          # (B,HW,C)
    gath = np.take_along_axis(
        hwdesc, ids16.reshape(B, N * 16)[:, :, None], axis=1
    ).reshape(B, N, 16, C)
    vm16 = np.einsum("bnc,bnjc->bnj", kp1_desc, gath).astype(f)     # (B,N,16)

    TOL = 2e-3
    PATCH_W = 48
    neg_scores = np.empty((B, N, NUM_NEG), f)
    repair = []
    for b in range(B):
        for n in range(N):
            if bad[b, n]:
                repair.append((b, n))
                continue
            cv = srt[b, n, :PATCH_W].copy()
            uq, inv, cnts = np.unique(
                ids16[b, n], return_index=True, return_counts=True
            )
            vms = vm16[b, n][inv]
            lo = cv[-1] - TOL
            ok = True
            for v, cnt in zip(vms, cnts):
                if v < lo:
                    continue
                j = np.argmin(np.abs(cv - v))
                if abs(cv[j] - v) > TOL:
                    ok = False
                    break
                cv[j] -= f(2.5) * cnt
            if not ok:
                repair.append((b, n))
                continue
            merged = np.sort(np.concatenate([cv, srt[b, n, PATCH_W:]]))[::-1]
            neg_scores[b, n] = merged[:NUM_NEG]

    if repair:
        for b, n in repair:
            row = hwdesc[b] @ kp1_desc[b, n]                        # (HW,)
            np.subtract.at(row, ids16[b, n], f(2.5))
            neg_scores[b, n] = np.sort(row)[::-1][:NUM_NEG]

    neg = f(2.0) - f(2.0) * neg_scores                              # ascending dsim
    fos = np.mean(
        np.maximum(pos[..., None] - neg + f(MARGIN), f(0.0)) ** 2
    ).astype(f)

    # ---------------- sos (exact, host: O(N^2 C) ~ 3% of total FLOPs) ----
    def top8_ids(desc, mask):
        out = np.empty((B, N, SOS_NEG), np.int64)
        for b in range(B):
            sim = f(2.0) - f(2.0) * (desc[b] @ desc[b].T) + f(5.0) * mask[b]
            out[b] = np.argsort(sim, axis=-1, kind="stable")[:, :SOS_NEG]
        return out

    k_idsF = top8_ids(kp1_desc, kp1_mask)
    w_idsF = top8_ids(w_kp1_desc, w_kp1_mask)

    kd = np.take_along_axis(
        kp1_desc, k_idsF.reshape(B, N * 8)[:, :, None], axis=1
    ).reshape(B, N, 8, C)
    wd = np.take_along_axis(
        w_kp1_desc, w_idsF.reshape(B, N * 8)[:, :, None], axis=1
    ).reshape(B, N, 8, C)
    a = f(2.0) - f(2.0) * np.einsum("bnc,bnkc->bnk", kp1_desc, kd)
    bb = f(2.0) - f(2.0) * np.einsum("bnc,bnkc->bnk", w_kp1_desc, wd)
    sv = (a - bb).astype(f)
    sos = np.mean(np.sqrt(np.sum(sv * sv, axis=-1))).astype(f)

    return np.asarray(fos + sos, dtype=np.float32)


# revision 40
# speedup vs baseline: 1.0397x; 1.0085x over previous
"""Trainium2 Bass kernel for nn_HardQuadTripletSOSRLoss.

Sharding: 8 cores = 2 batches x 4 HW-shards (4096 grid cells each).

Device work per core (b, s) — the dominant retrieval stage (>97% of FLOPs):
  - PE: bf16 matmuls, dsim scores kp1_desc[b] @ desc2f[b, shard]^T as
    4 row-tiles x 2 chunks of 2048 cols (4 x 512-col PSUM banks each),
    with warm-up matmuls to ramp the PE p-state while DMAs land.
  - ACT: drains each PSUM chunk to SBUF as fp16 (the conversion enables
    the DVE's packed 2-elem/cycle mode downstream).
  - DVE: two packed pairwise-max halvings (2048->1024->512) + max8 ->
    top-8-of-quads candidate values per chunk.
Host: bilinear descriptor sampling, grid geometry, masks, the small
O(N^2 C) k_sim/w_sim SOS stage, candidate merge with an exactness
certificate + per-row repair, final loss.
"""

import numpy as np
import ml_dtypes

import concourse.bass as bass
import concourse.mybir as mybir
import concourse.tile as tile
from concourse import bacc
from concourse.bass_utils import run_bass_kernel_spmd

# ---- problem constants (hardcoded per contract) ----
B, N, C, H, W = 2, 512, 128, 128, 128
HW = H * W
GS = 8
NUM_NEG = 16
SOS_NEG = 8
MARGIN = 1.0
NSHARD = 4
SHW = HW // NSHARD          # 4096 cells per shard
RT = N // 128               # 4 row tiles
CHUNK = 2048
NCH = SHW // CHUNK          # 2 chunks per row-tile

F32 = mybir.dt.float32
F16 = mybir.dt.float16
BF16 = mybir.dt.bfloat16
BF = ml_dtypes.bfloat16

_NC_CACHE = {}
LAST_RESULTS = None  # BassKernelResults of most recent device run (for test.py)


def _build_nc():
    nc = bacc.Bacc("TRN2", target_bir_lowering=False, debug=False, num_devices=8)

    lhsT = nc.dram_tensor("lhsT", [C, N], BF16, kind="ExternalInput")
    rhs = nc.dram_tensor("rhs", [C, SHW], BF16, kind="ExternalInput")
    cand = nc.dram_tensor("cand", [128, RT * NCH * 8], F16, kind="ExternalOutput")

    with tile.TileContext(nc) as tc:
        with (
            tc.tile_pool(name="const", bufs=1) as cpool,
            tc.tile_pool(name="cv", bufs=4) as cvpool,
            tc.tile_pool(name="h1", bufs=3) as h1pool,
            tc.tile_pool(name="h2", bufs=3) as h2pool,
            tc.tile_pool(name="psum", bufs=2, space="PSUM") as pspool,
        ):
            warm = cpool.tile([128, 512], BF16, tag="warm")
            nc.vector.memset(warm[:], 0)
            lhsT_sb = cpool.tile([C, N], BF16, tag="lhsT")
            rhs_sb = cpool.tile([C, SHW], BF16, tag="rhs")
            # spread streams over the idle DGE queues; chunk0's four 512-col
            # streams match its four matmuls so they start as data arrives
            nc.sync.dma_start(lhsT_sb[:], lhsT[:, :])
            nc.sync.dma_start(rhs_sb[:, 0:512], rhs[:, 0:512])
            nc.sync.dma_start(rhs_sb[:, 512:1024], rhs[:, 512:1024])
            nc.scalar.dma_start(rhs_sb[:, 1024:1536], rhs[:, 1024:1536])
            nc.scalar.dma_start(rhs_sb[:, 1536:2048], rhs[:, 1536:2048])
            for lo, hi in ((2048, 2560), (2560, 3072), (3072, 3584), (3584, 4096)):
                nc.gpsimd.dma_start(rhs_sb[:, lo:hi], rhs[:, lo:hi])

            cnd = cpool.tile([128, RT * NCH * 8], F16, tag="cnd")
            MX = mybir.AluOpType.max

            # PE p-state warm-up on zeros while the real inputs stream in
            Gw = pspool.tile([128, 2048], F32, tag="ps")
            for _ in range(3):
                nc.tensor.matmul(
                    Gw[:, 0:512], warm[:, 0:128], warm[:], start=True, stop=True
                )

            for i in range(RT * NCH):
                rt, ch = divmod(i, NCH)
                G = pspool.tile([128, 2048], F32, tag="ps")
                for q in range(4):
                    nc.tensor.matmul(
                        G[:, q * 512 : (q + 1) * 512],
                        lhsT_sb[:, rt * 128 : (rt + 1) * 128],
                        rhs_sb[:, ch * 2048 + q * 512 : ch * 2048 + (q + 1) * 512],
                        start=True,
                        stop=True,
                    )
                cv = cvpool.tile([128, 2048], F16, tag="cv")
                nc.scalar.copy(cv[:], G[:])
                h1 = h1pool.tile([128, 1024], F16, tag="h1")
                nc.vector.tensor_tensor(h1[:], cv[:, 0:1024], cv[:, 1024:2048], MX)
                h2 = h2pool.tile([128, 512], F16, tag="h2")
                nc.vector.tensor_tensor(h2[:], h1[:, 0:512], h1[:, 512:1024], MX)
                nc.vector.max(cnd[:, i * 8 : (i + 1) * 8], h2[:])
                if i == 3:
                    # ship the first half mid-stream so the final DMA's
                    # descriptor-gen is off the critical tail
                    nc.sync.dma_start(cand[:, 0:32], cnd[:, 0:32])

            nc.sync.dma_start(cand[:, 32:64], cnd[:, 32:64])

    nc.compile()
    return nc


def _get_nc():
    if "nc" not in _NC_CACHE:
        _NC_CACHE["nc"] = _build_nc()
    return _NC_CACHE["nc"]


# ---------------- host-side helpers (all float32, mirror reference) ----------


def _sample_descriptors(desc2, kp):
    """Bilinear sample of desc2 (B,C,H,W) at image-space (y,x) kp, L2-normed."""
    b, c, h, w = desc2.shape
    f = np.float32
    y = np.clip(kp[..., 0] / f(GS) - f(0.5), f(0.0), f(h - 1.0)).astype(f)
    x = np.clip(kp[..., 1] / f(GS) - f(0.5), f(0.0), f(w - 1.0)).astype(f)
    y0 = np.clip(np.floor(y), 0, h - 2).astype(np.int64)
    x0 = np.clip(np.floor(x), 0, w - 2).astype(np.int64)
    wy = (y - y0.astype(f))[..., None]
    wx = (x - x0.astype(f))[..., None]
    dmap = desc2.transpose(0, 2, 3, 1).reshape(b, h * w, c)

    def g(yi, xi):
        idx = yi * w + xi
        return np.take_along_axis(dmap, idx[..., None], axis=1)

    v = (
        g(y0, x0) * (1 - wy) * (1 - wx)
        + g(y0, x0 + 1) * (1 - wy) * wx
        + g(y0 + 1, x0) * wy * (1 - wx)
        + g(y0 + 1, x0 + 1) * wy * wx
    )
    n = np.sqrt(np.sum(v * v, axis=-1, keepdims=True)).astype(f)
    return (v / (n + f(1e-8))).astype(f)


def _nearest4(pts):
    """Flat ids (..., 4) of the 4 nearest grid-cell centers, matching the
    reference's top_k over all HW cells (ties -> lower flat id)."""
    f = np.float32
    y = pts[..., 0]
    x = pts[..., 1]
    cy = np.clip(np.floor(y / f(GS)).astype(np.int64), 0, H - 1)
    cx = np.clip(np.floor(x / f(GS)).astype(np.int64), 0, W - 1)
    by = np.clip(cy - 2, 0, H - 5)
    bx = np.clip(cx - 2, 0, W - 5)
    offs = np.arange(5, dtype=np.int64)
    iy = by[..., None] + offs          # (..., 5)
    ix = bx[..., None] + offs
    cyc = (f(GS) * iy + f(GS / 2.0)).astype(f)
    cxc = (f(GS) * ix + f(GS / 2.0)).astype(f)
    dy = y[..., None] - cyc
    dx = x[..., None] - cxc
    d2 = (dy * dy)[..., :, None] + (dx * dx)[..., None, :]   # (..., 5, 5)
    ids = iy[..., :, None] * W + ix[..., None, :]
    d2 = d2.reshape(d2.shape[:-2] + (25,))
    ids = ids.reshape(ids.shape[:-2] + (25,))
    # candidates are flat-id ascending, so a stable sort on d2 reproduces
    # top_k's lower-index tie-break
    order = np.argsort(d2, axis=-1, kind="stable")[..., :4]
    return np.take_along_axis(ids, order, axis=-1)


def _warp(p, Hm):
    f = np.float32
    xy = p[..., ::-1]
    ph = np.concatenate([xy, np.ones_like(xy[..., :1])], axis=-1)
    wp = np.einsum("bij,bmj->bmi", Hm, ph).astype(f)
    wp = wp[..., :2] / (wp[..., 2:3] + f(1e-8))
    return wp[..., ::-1].astype(f)


def _centers(ids):
    f = np.float32
    yy = (ids // W).astype(f) * f(GS) + f(GS / 2.0)
    xx = (ids % W).astype(f) * f(GS) + f(GS / 2.0)
    return np.stack([yy, xx], axis=-1)


def kernel(kp1, w_kp1, kp1_desc, desc2, homo12):
    global LAST_RESULTS
    import os

    f = np.float32
    kp1 = np.asarray(kp1, f)
    w_kp1 = np.asarray(w_kp1, f)
    kp1_desc = np.asarray(kp1_desc, f)
    desc2 = np.asarray(desc2, f)
    homo12 = np.asarray(homo12, f)

    # ---------------- host geometry / small tensors ----------------
    w_kp1_desc = _sample_descriptors(desc2, w_kp1)                  # (B,N,C)
    pos = f(2.0) - f(2.0) * np.einsum("bnc,bnc->bn", kp1_desc, w_kp1_desc)

    cell4 = _nearest4(kp1)                                          # (B,N,4)
    kp1_cells = _centers(cell4.reshape(B, 4 * N))                   # (B,4N,2)
    warped = _warp(kp1_cells, homo12)                               # (B,4N,2)
    wcc = _nearest4(warped)                                         # (B,4N,4)
    ids16 = wcc.reshape(B, N, 16)                                   # neigh cells
    cell4_w = _nearest4(w_kp1)                                      # (B,N,4)

    # kp1_mask[n,n'] = #coinciding cells between cell4[n] and cell4[n']
    eqk = cell4[:, :, :, None, None] == cell4[:, None, None, :, :]
    kp1_mask = eqk.sum(axis=(2, 4)).astype(f)                       # (B,N,N)
    # w_kp1_mask[n,n'] = #coincidences between ids16[n] and cell4_w[n']
    eqw = ids16[:, :, :, None, None] == cell4_w[:, None, None, :, :]
    w_kp1_mask = eqw.sum(axis=(2, 4)).astype(f)                     # (B,N,N)

    # ---------------- device run (dsim retrieval) ----------------
    nc = _get_nc()
    in_maps = []
    desc2_flat = np.ascontiguousarray(desc2.reshape(B, C, HW))
    for b in range(B):
        lhsT_b = np.ascontiguousarray(kp1_desc[b].T.astype(BF))
        for s in range(NSHARD):
            in_maps.append(
                {
                    "lhsT": lhsT_b,
                    "rhs": np.ascontiguousarray(
                        desc2_flat[b][:, s * SHW : (s + 1) * SHW].astype(BF)
                    ),
                }
            )
    want_trace = bool(int(os.environ.get("KT_TRACE", "0")))
    try:
        res = run_bass_kernel_spmd(
            nc, in_maps, core_ids=list(range(8)), trace=want_trace
        )
    except ModuleNotFoundError:
        res = run_bass_kernel_spmd(nc, in_maps, core_ids=list(range(8)), trace=False)
    LAST_RESULTS = res
    results = res.results

    # cand_all[b, n, chunk(8 per batch-row), 8]; chunk s*2+ch covers shard-s
    # cells [ch*2048, (ch+1)*2048)
    NCHB = NSHARD * NCH                                             # 8
    cand_all = np.empty((B, N, NCHB, 8), f)
    for ci, (b, s) in enumerate((b, s) for b in range(B) for s in range(NSHARD)):
        cm = np.asarray(results[ci]["cand"], np.float32)            # (128, 64)
        for rt in range(RT):
            for ch in range(NCH):
                i = rt * NCH + ch
                cand_all[b, rt * 128 : (rt + 1) * 128, s * NCH + ch, :] = cm[
                    :, i * 8 : (i + 1) * 8
                ]

    # ---------------- fos: merge per-chunk candidates ----------------
    flat = cand_all.reshape(B, N, NCHB * 8)
    chunk_min = cand_all[..., 7]                                    # (B,N,8)
    srt = np.sort(flat, axis=-1)[..., ::-1]                         # desc
    thr32 = srt[..., 31]
    CERT_EPS = f(2e-3)
    bad = (chunk_min >= thr32[..., None] - CERT_EPS).any(axis=-1)

    # host raw scores of masked cells (for value-matched patching)
    hwdesc = desc2_flat.transpose(0, 2, 1)                          # (B,HW,C)
    gath = np.take_along_axis(
        hwdesc, ids16.reshape(B, N * 16)[:, :, None], axis=1
    ).reshape(B, N, 16, C)
    vm16 = np.einsum("bnc,bnjc->bnj", kp1_desc, gath).astype(f)     # (B,N,16)

    TOL = 2e-3
    PATCH_W = 48
    neg_scores = np.empty((B, N, NUM_NEG), f)
    repair = []
    for b in range(B):
        for n in range(N):
            if bad[b, n]:
                repair.append((b, n))
                continue
            cv = srt[b, n, :PATCH_W].copy()
            uq, inv, cnts = np.unique(
                ids16[b, n], return_index=True, return_counts=True
            )
            vms = vm16[b, n][inv]
            lo = cv[-1] - TOL
            ok = True
            for v, cnt in zip(vms, cnts):
                if v < lo:
                    continue
                j = np.argmin(np.abs(cv - v))
                if abs(cv[j] - v) > TOL:
                    ok = False
                    break
                cv[j] -= f(2.5) * cnt
            if not ok:
                repair.append((b, n))
                continue
            merged = np.sort(np.concatenate([cv, srt[b, n, PATCH_W:]]))[::-1]
            neg_scores[b, n] = merged[:NUM_NEG]

    if repair:
        for b, n in repair:
            row = hwdesc[b] @ kp1_desc[b, n]                        # (HW,)
            np.subtract.at(row, ids16[b, n], f(2.5))
            neg_scores[b, n] = np.sort(row)[::-1][:NUM_NEG]

    neg = f(2.0) - f(2.0) * neg_scores                              # ascending dsim
    fos = np.mean(
        np.maximum(pos[..., None] - neg + f(MARGIN), f(0.0)) ** 2
    ).astype(f)

    # ---------------- sos (exact, host: O(N^2 C) ~ 3% of total FLOPs) ----
    def top8_ids(desc, mask):
        out = np.empty((B, N, SOS_NEG), np.int64)
        for b in range(B):
            sim = f(2.0) - f(2.0) * (desc[b] @ desc[b].T) + f(5.0) * mask[b]
            out[b] = np.argsort(sim, axis=-1, kind="stable")[:, :SOS_NEG]
        return out

    k_idsF = top8_ids(kp1_desc, kp1_mask)
    w_idsF = top8_ids(w_kp1_desc, w_kp1_mask)

    kd = np.take_along_axis(
        kp1_desc, k_idsF.reshape(B, N * 8)[:, :, None], axis=1
    ).reshape(B, N, 8, C)
    wd = np.take_along_axis(
        w_kp1_desc, w_idsF.reshape(B, N * 8)[:, :, None], axis=1
    ).reshape(B, N, 8, C)
    a = f(2.0) - f(2.0) * np.einsum("bnc,bnkc->bnk", kp1_desc, kd)
    bb = f(2.0) - f(2.0) * np.einsum("bnc,bnkc->bnk", w_kp1_desc, wd)
    sv = (a - bb).astype(f)
    sos = np.mean(np.sqrt(np.sum(sv * sv, axis=-1))).astype(f)

    return np.asarray(fos + sos, dtype=np.float32)


# revision 41
# speedup vs baseline: 1.0403x; 1.0006x over previous
"""Trainium2 Bass kernel for nn_HardQuadTripletSOSRLoss.

Sharding: 8 cores = 2 batches x 4 HW-shards (4096 grid cells each).

Device work per core (b, s) — the dominant retrieval stage (>97% of FLOPs):
  - PE: bf16 matmuls, dsim scores kp1_desc[b] @ desc2f[b, shard]^T as
    4 row-tiles x 2 chunks of 2048 cols (4 x 512-col PSUM banks each),
    with warm-up matmuls to ramp the PE p-state while DMAs land.
  - ACT: drains each PSUM chunk to SBUF as fp16 (the conversion enables
    the DVE's packed 2-elem/cycle mode downstream).
  - DVE: two packed pairwise-max halvings (2048->1024->512) + max8 ->
    top-8-of-quads candidate values per chunk.
Host: bilinear descriptor sampling, grid geometry, masks, the small
O(N^2 C) k_sim/w_sim SOS stage, candidate merge with an exactness
certificate + per-row repair, final loss.
"""

import numpy as np
import ml_dtypes

import concourse.bass as bass
import concourse.mybir as mybir
import concourse.tile as tile
from concourse import bacc
from concourse.bass_utils import run_bass_kernel_spmd

# ---- problem constants (hardcoded per contract) ----
B, N, C, H, W = 2, 512, 128, 128, 128
HW = H * W
GS = 8
NUM_NEG = 16
SOS_NEG = 8
MARGIN = 1.0
NSHARD = 4
SHW = HW // NSHARD          # 4096 cells per shard
RT = N // 128               # 4 row tiles
CHUNK = 2048
NCH = SHW // CHUNK          # 2 chunks per row-tile

F32 = mybir.dt.float32
F16 = mybir.dt.float16
BF16 = mybir.dt.bfloat16
BF = ml_dtypes.bfloat16

_NC_CACHE = {}
LAST_RESULTS = None  # BassKernelResults of most recent device run (for test.py)


def _build_nc():
    nc = bacc.Bacc("TRN2", target_bir_lowering=False, debug=False, num_devices=8)

    lhsT = nc.dram_tensor("lhsT", [C, N], BF16, kind="ExternalInput")
    rhs = nc.dram_tensor("rhs", [C, SHW], BF16, kind="ExternalInput")
    cand = nc.dram_tensor("cand", [128, RT * NCH * 8], F16, kind="ExternalOutput")

    with tile.TileContext(nc) as tc:
        with (
            tc.tile_pool(name="const", bufs=1) as cpool,
            tc.tile_pool(name="cv", bufs=4) as cvpool,
            tc.tile_pool(name="h1", bufs=3) as h1pool,
            tc.tile_pool(name="h2", bufs=3) as h2pool,
            tc.tile_pool(name="psum", bufs=2, space="PSUM") as pspool,
        ):
            warm = cpool.tile([128, 512], BF16, tag="warm")
            nc.vector.memset(warm[:], 0)
            lhsT_sb = cpool.tile([C, N], BF16, tag="lhsT")
            rhs_sb = cpool.tile([C, SHW], BF16, tag="rhs")
            # spread streams over the idle DGE queues; chunk0's four 512-col
            # streams match its four matmuls so they start as data arrives
            nc.sync.dma_start(lhsT_sb[:], lhsT[:, :])
            nc.sync.dma_start(rhs_sb[:, 0:512], rhs[:, 0:512])
            nc.sync.dma_start(rhs_sb[:, 512:1024], rhs[:, 512:1024])
            nc.scalar.dma_start(rhs_sb[:, 1024:1536], rhs[:, 1024:1536])
            nc.scalar.dma_start(rhs_sb[:, 1536:2048], rhs[:, 1536:2048])
            for lo, hi in ((2048, 2560), (2560, 3072), (3072, 3584), (3584, 4096)):
                nc.gpsimd.dma_start(rhs_sb[:, lo:hi], rhs[:, lo:hi])

            cnd = cpool.tile([128, RT * NCH * 8], F16, tag="cnd")
            MX = mybir.AluOpType.max

            # PE p-state warm-up on zeros while the real inputs stream in
            Gw = pspool.tile([128, 2048], F32, tag="ps")
            for _ in range(3):
                nc.tensor.matmul(
                    Gw[:, 0:512], warm[:, 0:128], warm[:], start=True, stop=True
                )

            for i in range(RT * NCH):
                rt, ch = divmod(i, NCH)
                G = pspool.tile([128, 2048], F32, tag="ps")
                for q in range(4):
                    nc.tensor.matmul(
                        G[:, q * 512 : (q + 1) * 512],
                        lhsT_sb[:, rt * 128 : (rt + 1) * 128],
                        rhs_sb[:, ch * 2048 + q * 512 : ch * 2048 + (q + 1) * 512],
                        start=True,
                        stop=True,
                    )
                if i == 0:
                    # chunk 0 drains on the DVE straight off PSUM (top-8
                    # singles): DVE starts at matmul-completion instead of
                    # waiting for an ACT conv, and the ACT spine shrinks to
                    # 7 chunks
                    v8d = cpool.tile([128, 8], F32, tag="v8d")
                    nc.vector.max(v8d[:], G[:])
                    nc.vector.tensor_copy(cnd[:, 0:8], v8d[:])
                    continue
                cv = cvpool.tile([128, 2048], F16, tag="cv")
                nc.scalar.copy(cv[:], G[:])
                h1 = h1pool.tile([128, 1024], F16, tag="h1")
                nc.vector.tensor_tensor(h1[:], cv[:, 0:1024], cv[:, 1024:2048], MX)
                h2 = h2pool.tile([128, 512], F16, tag="h2")
                nc.vector.tensor_tensor(h2[:], h1[:, 0:512], h1[:, 512:1024], MX)
                nc.vector.max(cnd[:, i * 8 : (i + 1) * 8], h2[:])
                if i == 3:
                    # ship the first half mid-stream so the final DMA's
                    # descriptor-gen is off the critical tail
                    nc.sync.dma_start(cand[:, 0:32], cnd[:, 0:32])

            nc.sync.dma_start(cand[:, 32:64], cnd[:, 32:64])

    nc.compile()
    return nc


def _get_nc():
    if "nc" not in _NC_CACHE:
        _NC_CACHE["nc"] = _build_nc()
    return _NC_CACHE["nc"]


# ---------------- host-side helpers (all float32, mirror reference) ----------


def _sample_descriptors(desc2, kp):
    """Bilinear sample of desc2 (B,C,H,W) at image-space (y,x) kp, L2-normed."""
    b, c, h, w = desc2.shape
    f = np.float32
    y = np.clip(kp[..., 0] / f(GS) - f(0.5), f(0.0), f(h - 1.0)).astype(f)
    x = np.clip(kp[..., 1] / f(GS) - f(0.5), f(0.0), f(w - 1.0)).astype(f)
    y0 = np.clip(np.floor(y), 0, h - 2).astype(np.int64)
    x0 = np.clip(np.floor(x), 0, w - 2).astype(np.int64)
    wy = (y - y0.astype(f))[..., None]
    wx = (x - x0.astype(f))[..., None]
    dmap = desc2.transpose(0, 2, 3, 1).reshape(b, h * w, c)

    def g(yi, xi):
        idx = yi * w + xi
        return np.take_along_axis(dmap, idx[..., None], axis=1)

    v = (
        g(y0, x0) * (1 - wy) * (1 - wx)
        + g(y0, x0 + 1) * (1 - wy) * wx
        + g(y0 + 1, x0) * wy * (1 - wx)
        + g(y0 + 1, x0 + 1) * wy * wx
    )
    n = np.sqrt(np.sum(v * v, axis=-1, keepdims=True)).astype(f)
    return (v / (n + f(1e-8))).astype(f)


def _nearest4(pts):
    """Flat ids (..., 4) of the 4 nearest grid-cell centers, matching the
    reference's top_k over all HW cells (ties -> lower flat id)."""
    f = np.float32
    y = pts[..., 0]
    x = pts[..., 1]
    cy = np.clip(np.floor(y / f(GS)).astype(np.int64), 0, H - 1)
    cx = np.clip(np.floor(x / f(GS)).astype(np.int64), 0, W - 1)
    by = np.clip(cy - 2, 0, H - 5)
    bx = np.clip(cx - 2, 0, W - 5)
    offs = np.arange(5, dtype=np.int64)
    iy = by[..., None] + offs          # (..., 5)
    ix = bx[..., None] + offs
    cyc = (f(GS) * iy + f(GS / 2.0)).astype(f)
    cxc = (f(GS) * ix + f(GS / 2.0)).astype(f)
    dy = y[..., None] - cyc
    dx = x[..., None] - cxc
    d2 = (dy * dy)[..., :, None] + (dx * dx)[..., None, :]   # (..., 5, 5)
    ids = iy[..., :, None] * W + ix[..., None, :]
    d2 = d2.reshape(d2.shape[:-2] + (25,))
    ids = ids.reshape(ids.shape[:-2] + (25,))
    # candidates are flat-id ascending, so a stable sort on d2 reproduces
    # top_k's lower-index tie-break
    order = np.argsort(d2, axis=-1, kind="stable")[..., :4]
    return np.take_along_axis(ids, order, axis=-1)


def _warp(p, Hm):
    f = np.float32
    xy = p[..., ::-1]
    ph = np.concatenate([xy, np.ones_like(xy[..., :1])], axis=-1)
    wp = np.einsum("bij,bmj->bmi", Hm, ph).astype(f)
    wp = wp[..., :2] / (wp[..., 2:3] + f(1e-8))
    return wp[..., ::-1].astype(f)


def _centers(ids):
    f = np.float32
    yy = (ids // W).astype(f) * f(GS) + f(GS / 2.0)
    xx = (ids % W).astype(f) * f(GS) + f(GS / 2.0)
    return np.stack([yy, xx], axis=-1)


def kernel(kp1, w_kp1, kp1_desc, desc2, homo12):
    global LAST_RESULTS
    import os

    f = np.float32
    kp1 = np.asarray(kp1, f)
    w_kp1 = np.asarray(w_kp1, f)
    kp1_desc = np.asarray(kp1_desc, f)
    desc2 = np.asarray(desc2, f)
    homo12 = np.asarray(homo12, f)

    # ---------------- host geometry / small tensors ----------------
    w_kp1_desc = _sample_descriptors(desc2, w_kp1)                  # (B,N,C)
    pos = f(2.0) - f(2.0) * np.einsum("bnc,bnc->bn", kp1_desc, w_kp1_desc)

    cell4 = _nearest4(kp1)                                          # (B,N,4)
    kp1_cells = _centers(cell4.reshape(B, 4 * N))                   # (B,4N,2)
    warped = _warp(kp1_cells, homo12)                               # (B,4N,2)
    wcc = _nearest4(warped)                                         # (B,4N,4)
    ids16 = wcc.reshape(B, N, 16)                                   # neigh cells
    cell4_w = _nearest4(w_kp1)                                      # (B,N,4)

    # kp1_mask[n,n'] = #coinciding cells between cell4[n] and cell4[n']
    eqk = cell4[:, :, :, None, None] == cell4[:, None, None, :, :]
    kp1_mask = eqk.sum(axis=(2, 4)).astype(f)                       # (B,N,N)
    # w_kp1_mask[n,n'] = #coincidences between ids16[n] and cell4_w[n']
    eqw = ids16[:, :, :, None, None] == cell4_w[:, None, None, :, :]
    w_kp1_mask = eqw.sum(axis=(2, 4)).astype(f)                     # (B,N,N)

    # ---------------- device run (dsim retrieval) ----------------
    nc = _get_nc()
    in_maps = []
    desc2_flat = np.ascontiguousarray(desc2.reshape(B, C, HW))
    for b in range(B):
        lhsT_b = np.ascontiguousarray(kp1_desc[b].T.astype(BF))
        for s in range(NSHARD):
            in_maps.append(
                {
                    "lhsT": lhsT_b,
                    "rhs": np.ascontiguousarray(
                        desc2_flat[b][:, s * SHW : (s + 1) * SHW].astype(BF)
                    ),
                }
            )
    want_trace = bool(int(os.environ.get("KT_TRACE", "0")))
    try:
        res = run_bass_kernel_spmd(
            nc, in_maps, core_ids=list(range(8)), trace=want_trace
        )
    except ModuleNotFoundError:
        res = run_bass_kernel_spmd(nc, in_maps, core_ids=list(range(8)), trace=False)
    LAST_RESULTS = res
    results = res.results

    # cand_all[b, n, chunk(8 per batch-row), 8]; chunk s*2+ch covers shard-s
    # cells [ch*2048, (ch+1)*2048)
    NCHB = NSHARD * NCH                                             # 8
    cand_all = np.empty((B, N, NCHB, 8), f)
    for ci, (b, s) in enumerate((b, s) for b in range(B) for s in range(NSHARD)):
        cm = np.asarray(results[ci]["cand"], np.float32)            # (128, 64)
        for rt in range(RT):
            for ch in range(NCH):
                i = rt * NCH + ch
                cand_all[b, rt * 128 : (rt + 1) * 128, s * NCH + ch, :] = cm[
                    :, i * 8 : (i + 1) * 8
                ]

    # ---------------- fos: merge per-chunk candidates ----------------
    flat = cand_all.reshape(B, N, NCHB * 8)
    chunk_min = cand_all[..., 7]                                    # (B,N,8)
    srt = np.sort(flat, axis=-1)[..., ::-1]                         # desc
    thr32 = srt[..., 31]
    CERT_EPS = f(2e-3)
    bad = (chunk_min >= thr32[..., None] - CERT_EPS).any(axis=-1)

    # host raw scores of masked cells (for value-matched patching)
    hwdesc = desc2_flat.transpose(0, 2, 1)                          # (B,HW,C)
    gath = np.take_along_axis(
        hwdesc, ids16.reshape(B, N * 16)[:, :, None], axis=1
    ).reshape(B, N, 16, C)
    vm16 = np.einsum("bnc,bnjc->bnj", kp1_desc, gath).astype(f)     # (B,N,16)

    TOL = 2e-3
    PATCH_W = 48
    neg_scores = np.empty((B, N, NUM_NEG), f)
    repair = []
    for b in range(B):
        for n in range(N):
            if bad[b, n]:
                repair.append((b, n))
                continue
            cv = srt[b, n, :PATCH_W].copy()
            uq, inv, cnts = np.unique(
                ids16[b, n], return_index=True, return_counts=True
            )
            vms = vm16[b, n][inv]
            lo = cv[-1] - TOL
            ok = True
            for v, cnt in zip(vms, cnts):
                if v < lo:
                    continue
                j = np.argmin(np.abs(cv - v))
                if abs(cv[j] - v) > TOL:
                    ok = False
                    break
                cv[j] -= f(2.5) * cnt
            if not ok:
                repair.append((b, n))
                continue
            merged = np.sort(np.concatenate([cv, srt[b, n, PATCH_W:]]))[::-1]
            neg_scores[b, n] = merged[:NUM_NEG]

    if repair:
        for b, n in repair:
            row = hwdesc[b] @ kp1_desc[b, n]                        # (HW,)
            np.subtract.at(row, ids16[b, n], f(2.5))
            neg_scores[b, n] = np.sort(row)[::-1][:NUM_NEG]

    neg = f(2.0) - f(2.0) * neg_scores                              # ascending dsim
    fos = np.mean(
        np.maximum(pos[..., None] - neg + f(MARGIN), f(0.0)) ** 2
    ).astype(f)

    # ---------------- sos (exact, host: O(N^2 C) ~ 3% of total FLOPs) ----
    def top8_ids(desc, mask):
        out = np.empty((B, N, SOS_NEG), np.int64)
        for b in range(B):
            sim = f(2.0) - f(2.0) * (desc[b] @ desc[b].T) + f(5.0) * mask[b]
            out[b] = np.argsort(sim, axis=-1, kind="stable")[:, :SOS_NEG]
        return out

    k_idsF = top8_ids(kp1_desc, kp1_mask)
    w_idsF = top8_ids(w_kp1_desc, w_kp1_mask)

    kd = np.take_along_axis(
        kp1_desc, k_idsF.reshape(B, N * 8)[:, :, None], axis=1
    ).reshape(B, N, 8, C)
    wd = np.take_along_axis(
        w_kp1_desc, w_idsF.reshape(B, N * 8)[:, :, None], axis=1
    ).reshape(B, N, 8, C)
    a = f(2.0) - f(2.0) * np.einsum("bnc,bnkc->bnk", kp1_desc, kd)
    bb = f(2.0) - f(2.0) * np.einsum("bnc,bnkc->bnk", w_kp1_desc, wd)
    sv = (a - bb).astype(f)
    sos = np.mean(np.sqrt(np.sum(sv * sv, axis=-1))).astype(f)

    return np.asarray(fos + sos, dtype=np.float32)
